# revision 1
# baseline (speedup 1.0000x reference)
"""KAN layer (Chebyshev order-7 on tanh(x)) as a Bass/Tile TRN2 kernel.

Math: out[b,o] = sum_{i,k} T_k(tanh(x[b,i])) * W[o,i,k] + bias[o],  k=0..7.

T_0 == 1, so the k=0 weight slice folds into an effective bias on the host:
bias_eff[o] = bias[o] + sum_i W[o,i,0]. The device contracts over the
remaining 7*1024 = 7168 (i,k) pairs.

Sharding: data-parallel over batch. Each of the 8 cores takes 512 batch
rows; every core holds the full weights. Per core this is a
[7168 x 512] basis (built on-chip from x) against [7168 x 1024] weights,
accumulated as out.T tiles [128(o) x 512(b)] across 8 PSUM banks with
fp32r matmuls (full PE rate at free-dim 512).
"""

import sys

sys.path.insert(0, "/opt/trn_rl_repo")

import numpy as np

import concourse.bass as bass  # noqa: F401  (engine types come via bacc)
import concourse.mybir as mybir
from concourse import bacc
from concourse.bass_utils import run_bass_kernel_spmd
from concourse.tile import TileContext

P = 128
N_CORES = 8
BATCH = 4096
B_CORE = BATCH // N_CORES  # 512
IN_F = 1024
OUT_F = 1024
KORD = 7  # Chebyshev T_1..T_7 (T_0 folded into bias)
N_ITILES = IN_F // P  # 8
N_OTILES = OUT_F // P  # 8
NSTEPS = N_ITILES * KORD  # 56 contraction steps of K=128

F32 = mybir.dt.float32
F32R = mybir.dt.float32r
ACT_COPY = mybir.ActivationFunctionType.Copy
ACT_TANH = mybir.ActivationFunctionType.Tanh
MULT = mybir.AluOpType.mult

_NC_CACHE = None


def _build():
    """Build + compile the single-core Bass program (SPMD across 8 cores)."""
    global _NC_CACHE
    if _NC_CACHE is not None:
        return _NC_CACHE

    nc = bacc.Bacc("TRN2", target_bir_lowering=False, debug=False)

    # xT[i, b] = x[b, i] for this core's batch slice.
    xT = nc.declare_dram_parameter("xT", [IN_F, B_CORE], F32, isOutput=False)
    # wT[it, k', p, o] = weights[o, it*128+p, k'+1]  (fp32 bits, fp32r view).
    wT = nc.declare_dram_parameter(
        "wT", [N_ITILES, KORD, P, OUT_F], F32R, isOutput=False
    )
    # biasT[p, ot] = bias_eff[ot*128 + p]
    biasT = nc.declare_dram_parameter("biasT", [P, N_OTILES], F32, isOutput=False)
    outT = nc.declare_dram_parameter("outT", [OUT_F, B_CORE], F32, isOutput=True)

    with TileContext(nc) as tc:
        with (
            tc.tile_pool(name="basis", bufs=1) as basis_pool,
            tc.tile_pool(name="chain", bufs=8) as chain_pool,
            tc.tile_pool(name="tmp", bufs=3) as tmp_pool,
            tc.tile_pool(name="raw", bufs=2) as raw_pool,
            tc.tile_pool(name="w", bufs=8) as w_pool,
            tc.tile_pool(name="osb", bufs=3) as osb_pool,
            tc.tile_pool(name="misc", bufs=1) as misc_pool,
            tc.tile_pool(name="psum", bufs=1, space="PSUM") as psum_pool,
        ):
            bias_sb = misc_pool.tile([P, N_OTILES], F32, name="bias_sb")
            nc.sync.dma_start(out=bias_sb, in_=biasT[:, :])

            # ---- Chebyshev basis: chain in fp32, fp32r copies for the PE ----
            # basis_r[it][j] = T_{j+1}(tanh(xT tile it)) as [128, 512] fp32r
            basis_r = []
            for it in range(N_ITILES):
                traw = raw_pool.tile([P, B_CORE], F32, tag="traw")
                nc.sync.dma_start(out=traw, in_=xT[it * P : (it + 1) * P, :])
                t = chain_pool.tile([P, B_CORE], F32, tag="chain")
                nc.scalar.activation(t, traw, ACT_TANH)

                tiles_r = []
                t1r = basis_pool.tile([P, B_CORE], F32R, name=f"b_{it}_0")
                nc.scalar.activation(t1r, t, ACT_COPY)
                tiles_r.append(t1r)

                prev, prev2 = t, None
                for k in range(2, KORD + 1):
                    tmp = tmp_pool.tile([P, B_CORE], F32, tag="tmp")
                    # tmp = (t * 2) * T_{k-1}
                    nc.vector.scalar_tensor_tensor(
                        out=tmp, in0=t, scalar=2.0, in1=prev, op0=MULT, op1=MULT
                    )
                    cur = chain_pool.tile([P, B_CORE], F32, tag="chain")
                    if k == 2:
                        nc.vector.tensor_scalar_sub(cur, tmp, 1.0)
                    else:
                        nc.vector.tensor_sub(cur, tmp, prev2)
                    ckr = basis_pool.tile([P, B_CORE], F32R, name=f"b_{it}_{k - 1}")
                    nc.scalar.activation(ckr, cur, ACT_COPY)
                    tiles_r.append(ckr)
                    prev2, prev = prev, cur
                basis_r.append(tiles_r)

            # ---- Matmul accumulation: out.T[ot] += w_s[:, ot].T @ basis_s ----
            psums = [
                psum_pool.tile([P, B_CORE], F32, name=f"ps_{ot}")
                for ot in range(N_OTILES)
            ]
            HALF = OUT_F // 2
            s = 0
            for it in range(N_ITILES):
                for k in range(KORD):
                    # split the weight fetch so the first 4 matmuls can
                    # start as soon as half the step's weights land
                    wa = w_pool.tile([P, HALF], F32R, tag="wa")
                    nc.sync.dma_start(out=wa, in_=wT[it, k, :, :HALF])
                    wb = w_pool.tile([P, HALF], F32R, tag="wb")
                    nc.sync.dma_start(out=wb, in_=wT[it, k, :, HALF:])
                    rhs = basis_r[it][k]
                    for ot in range(N_OTILES):
                        wt = wa if ot < 4 else wb
                        col = (ot % 4) * P
                        nc.tensor.matmul(
                            psums[ot],
                            lhsT=wt[:, col : col + P],
                            rhs=rhs,
                            start=(s == 0),
                            stop=(s == NSTEPS - 1),
                        )
                    s += 1

            # ---- bias add + store ----
            for ot in range(N_OTILES):
                osb = osb_pool.tile([P, B_CORE], F32, tag="osb")
                nc.scalar.activation(
                    osb,
                    psums[ot],
                    mybir.ActivationFunctionType.Identity,
                    bias=bias_sb[:, ot : ot + 1],
                    scale=1.0,
                )
                nc.sync.dma_start(out=outT[ot * P : (ot + 1) * P, :], in_=osb)

    nc.compile()
    _NC_CACHE = nc
    return _NC_CACHE


def _prep_inputs(x, weights, bias_param):
    x = np.asarray(x, dtype=np.float32)
    weights = np.asarray(weights, dtype=np.float32)
    bias_param = np.asarray(bias_param, dtype=np.float32)

    # [o, i, k] -> [it, k'=k-1, p, o], contiguous
    w4 = weights.transpose(1, 2, 0)[:, 1:, :]  # [i, 7, o]
    w4 = np.ascontiguousarray(
        w4.reshape(N_ITILES, P, KORD, OUT_F).transpose(0, 2, 1, 3)
    )

    bias_eff = bias_param + weights[:, :, 0].sum(axis=1)  # T_0 == 1 fold
    bias_t = np.ascontiguousarray(bias_eff.reshape(N_OTILES, P).T)  # [128, 8]

    in_maps = []
    for c in range(N_CORES):
        x_c = np.ascontiguousarray(x[c * B_CORE : (c + 1) * B_CORE].T)  # [1024, 512]
        in_maps.append({"xT": x_c, "wT": w4, "biasT": bias_t})
    return in_maps


def _run(x, weights, bias_param, **spmd_kwargs):
    nc = _build()
    in_maps = _prep_inputs(x, weights, bias_param)
    res = run_bass_kernel_spmd(nc, in_maps, core_ids=list(range(N_CORES)), **spmd_kwargs)
    out = np.empty((BATCH, OUT_F), dtype=np.float32)
    for c in range(N_CORES):
        out[c * B_CORE : (c + 1) * B_CORE] = res.results[c]["outT"].T
    return out, res


def kernel(x, weights, bias_param):
    out, _ = _run(x, weights, bias_param)
    return out



# revision 5
# speedup vs baseline: 1.1295x; 1.1295x over previous
"""KAN layer (Chebyshev order-7 on tanh(x)) as a Bass/Tile TRN2 kernel.

Math: out[b,o] = sum_{i,k} T_k(tanh(x[b,i])) * W[o,i,k] + bias[o],  k=0..7.

T_0 == 1, so the k=0 weight slice folds into an effective bias on the host:
bias_eff[o] = bias[o] + sum_i W[o,i,0]. The device contracts over the
remaining 7*1024 = 7168 (i,k) pairs.

Sharding: data-parallel over batch. Each of the 8 cores takes 512 batch
rows; every core holds the full weights. Per core this is a
[7168 x 512] basis (built on-chip from x) against [7168 x 1024] weights,
accumulated as out.T tiles [128(o) x 512(b)] across 8 PSUM banks.

The whole device pipeline runs in fp16 (weights, basis chain, matmul
operands, stored output) — fp16 matmuls stream at the full 1 row/cycle PE
rate, fp16 halves all DMA traffic, and the Chebyshev recurrence in fp16
keeps rel err ~1.4e-3 (gate is 2e-2). PSUM accumulation stays fp32; the
final output is cast back to fp32 on the host.
"""

import sys

sys.path.insert(0, "/opt/trn_rl_repo")

import numpy as np

import concourse.bass as bass  # noqa: F401  (engine types come via bacc)
import concourse.mybir as mybir
from concourse import bacc
from concourse.bass_utils import run_bass_kernel_spmd
from concourse.tile import TileContext

P = 128
N_CORES = 8
BATCH = 4096
B_CORE = BATCH // N_CORES  # 512
IN_F = 1024
OUT_F = 1024
KORD = 7  # Chebyshev T_1..T_7 (T_0 folded into bias)
N_ITILES = IN_F // P  # 8
N_OTILES = OUT_F // P  # 8
NSTEPS = N_ITILES * KORD  # 56 contraction steps of K=128
NWARM = 64  # PE p-state warmup matmuls covering the DMA-latency head

F32 = mybir.dt.float32
F16 = mybir.dt.float16
ACT_TANH = mybir.ActivationFunctionType.Tanh
ACT_IDENT = mybir.ActivationFunctionType.Identity
MULT = mybir.AluOpType.mult
ADD = mybir.AluOpType.add
SUB = mybir.AluOpType.subtract

_NC_CACHE = None


def _build():
    """Build + compile the single-core Bass program (SPMD across 8 cores)."""
    global _NC_CACHE
    if _NC_CACHE is not None:
        return _NC_CACHE

    nc = bacc.Bacc("TRN2", target_bir_lowering=False, debug=False)

    # xT[i, b] = x[b, i] for this core's batch slice (fp16).
    xT = nc.declare_dram_parameter("xT", [IN_F, B_CORE], F16, isOutput=False)
    # wT[it, p, k'*1024 + o] = weights[o, it*128+p, k'+1] (fp16).
    wT = nc.declare_dram_parameter(
        "wT", [N_ITILES, P, KORD * OUT_F], F16, isOutput=False
    )
    # biasT[p, ot] = bias_eff[ot*128 + p]
    biasT = nc.declare_dram_parameter("biasT", [P, N_OTILES], F32, isOutput=False)
    # outT[ot, p, b] = out[b, ot*128 + p] (fp16; host casts to fp32)
    outT = nc.declare_dram_parameter("outT", [N_OTILES, P, B_CORE], F16, isOutput=True)

    with TileContext(nc) as tc:
        with (
            tc.tile_pool(name="basis", bufs=1) as basis_pool,
            tc.tile_pool(name="x", bufs=1) as x_pool,
            tc.tile_pool(name="w0", bufs=1) as w0_pool,
            tc.tile_pool(name="wbig", bufs=3) as wbig_pool,
            tc.tile_pool(name="chain", bufs=3) as chain_pool,
            tc.tile_pool(name="osb", bufs=8) as osb_pool,
            tc.tile_pool(name="misc", bufs=1) as misc_pool,
            tc.tile_pool(name="psum", bufs=1, space="PSUM") as psum_pool,
        ):
            psums = [
                psum_pool.tile([P, B_CORE], F32, name=f"ps_{ot}")
                for ot in range(N_OTILES)
            ]

            # -- t=0: scratch memset, dummy tanh (pulls the ACT table load
            #    off the critical path), PE warmup during the DMA-latency
            #    head so the tensor engine p-state is hot when real matmuls
            #    arrive.
            scratch = misc_pool.tile([P, 64], F16, name="scratch")
            nc.vector.memset(scratch, 0.0)
            dumout = misc_pool.tile([P, 1], F32, name="dumout")
            nc.scalar.activation(dumout, scratch[:, :1], ACT_TANH)

            bias_sb = misc_pool.tile([P, N_OTILES], F32, name="bias_sb")

            # -- DMA issue order == DMA service order (single SP queue).
            x_tiles = [
                x_pool.tile([P, B_CORE], F16, name=f"x_{it}")
                for it in range(N_ITILES)
            ]
            w0_tiles = [
                w0_pool.tile([P, OUT_F], F16, name=f"w0_{k}") for k in range(KORD)
            ]
            wbig_tiles = [None] * N_ITILES

            # x0 lands in two batch-halves so the first tanh + first matmuls
            # start one DMA-half earlier.
            H = B_CORE // 2
            nc.sync.dma_start(out=x_tiles[0][:, :H], in_=xT[0:P, :H])
            nc.sync.dma_start(out=x_tiles[0][:, H:], in_=xT[0:P, H:])
            nc.sync.dma_start(out=bias_sb, in_=biasT[:, :])
            for k in range(2):
                nc.sync.dma_start(
                    out=w0_tiles[k], in_=wT[0, :, k * OUT_F : (k + 1) * OUT_F]
                )
            nc.sync.dma_start(out=x_tiles[1], in_=xT[P : 2 * P, :])
            for k in range(2, KORD):
                nc.sync.dma_start(
                    out=w0_tiles[k], in_=wT[0, :, k * OUT_F : (k + 1) * OUT_F]
                )
            nc.sync.dma_start(out=x_tiles[2], in_=xT[2 * P : 3 * P, :])
            nc.sync.dma_start(out=x_tiles[3], in_=xT[3 * P : 4 * P, :])
            for it in range(1, N_ITILES):
                wb = wbig_pool.tile([P, KORD * OUT_F], F16, tag="wbig")
                nc.sync.dma_start(out=wb, in_=wT[it, :, :])
                wbig_tiles[it] = wb
                if it + 3 < N_ITILES:
                    nc.sync.dma_start(
                        out=x_tiles[it + 3],
                        in_=xT[(it + 3) * P : (it + 4) * P, :],
                    )

            # -- PE warmup: tiny self-matmuls on the zero scratch tile into a
            #    PSUM region later overwritten by the real start=True group.
            for i in range(NWARM):
                nc.tensor.matmul(
                    psums[0][:64, :64],
                    lhsT=scratch,
                    rhs=scratch,
                    start=True,
                    stop=True,
                )

            # -- Chebyshev basis in fp16: t1 = tanh(x); v = 2*t1;
            #    t2 = 2*t1^2 - 1; t_k = v*t_{k-1} - t_{k-2}.
            basis = []  # basis[it][k'] = T_{k'+1} tile [128, 512] fp16
            for it in range(N_ITILES):
                t1 = basis_pool.tile([P, B_CORE], F16, name=f"b_{it}_0")
                nc.scalar.activation(t1, x_tiles[it], ACT_TANH)
                v = chain_pool.tile([P, B_CORE], F16, tag="v")
                nc.vector.tensor_scalar_mul(v, t1, 2.0)
                m = chain_pool.tile([P, B_CORE], F16, tag="m")
                nc.vector.tensor_tensor(out=m, in0=t1, in1=t1, op=MULT)
                t2 = basis_pool.tile([P, B_CORE], F16, name=f"b_{it}_1")
                nc.vector.tensor_scalar(
                    out=t2, in0=m, scalar1=2.0, scalar2=-1.0, op0=MULT, op1=ADD
                )
                tiles = [t1, t2]
                for k in range(3, KORD + 1):
                    mk = chain_pool.tile([P, B_CORE], F16, tag="m")
                    nc.vector.tensor_tensor(out=mk, in0=v, in1=tiles[-1], op=MULT)
                    tk = basis_pool.tile([P, B_CORE], F16, name=f"b_{it}_{k - 1}")
                    nc.vector.tensor_tensor(out=tk, in0=mk, in1=tiles[-2], op=SUB)
                    tiles.append(tk)
                basis.append(tiles)

            # -- Matmul accumulation: out.T[ot] += w_s[:, ot].T @ basis_s
            def lhsT_for(it, k, ot):
                if it == 0:
                    return w0_tiles[k][:, ot * P : (ot + 1) * P]
                col = k * OUT_F + ot * P
                return wbig_tiles[it][:, col : col + P]

            def emit_tail(ot, on_act):
                osb = osb_pool.tile([P, B_CORE], F16, tag="osb")
                if on_act:
                    nc.scalar.activation(
                        osb, psums[ot], ACT_IDENT,
                        bias=bias_sb[:, ot : ot + 1], scale=1.0,
                    )
                else:
                    nc.vector.tensor_scalar_add(osb, psums[ot], bias_sb[:, ot : ot + 1])
                nc.sync.dma_start(out=outT[ot, :, :], in_=osb)

            # Steps 0..48: all otiles interleaved (step-major keeps the basis
            # pipeline just ahead of the PE).
            steps = [(it, k) for it in range(N_ITILES) for k in range(KORD)]
            for s, (it, k) in enumerate(steps[: NSTEPS - KORD]):
                rhs = basis[it][k]
                for ot in range(N_OTILES):
                    nc.tensor.matmul(
                        psums[ot], lhsT=lhsT_for(it, k, ot), rhs=rhs,
                        start=(s == 0), stop=False,
                    )

            # Final 7 steps otile-staggered: otile o finishes ~7*(7-o) matmuls
            # before the end, so its bias-add + store hide under the
            # remaining matmuls; only ot7's tail is exposed.
            for ot in range(N_OTILES):
                for j, (it, k) in enumerate(steps[NSTEPS - KORD :]):
                    nc.tensor.matmul(
                        psums[ot], lhsT=lhsT_for(it, k, ot), rhs=basis[it][k],
                        start=False, stop=(j == KORD - 1),
                    )
                emit_tail(ot, on_act=(ot % 2 == 1))

    nc.compile()
    _NC_CACHE = nc
    return _NC_CACHE


def _prep_inputs(x, weights, bias_param):
    x = np.asarray(x, dtype=np.float32)
    weights = np.asarray(weights, dtype=np.float32)
    bias_param = np.asarray(bias_param, dtype=np.float32)

    # [o, i, k] -> [it, p, k'*1024 + o] fp16
    w4 = weights.transpose(1, 2, 0)[:, 1:, :]  # [i, 7, o]
    wt = np.ascontiguousarray(
        w4.reshape(N_ITILES, P, KORD * OUT_F).astype(np.float16)
    )

    # exact T_0 fold in float64
    bias_eff = (
        bias_param.astype(np.float64) + weights[:, :, 0].astype(np.float64).sum(axis=1)
    ).astype(np.float32)
    bias_t = np.ascontiguousarray(bias_eff.reshape(N_OTILES, P).T)  # [128, 8]

    in_maps = []
    for c in range(N_CORES):
        x_c = np.ascontiguousarray(
            x[c * B_CORE : (c + 1) * B_CORE].T.astype(np.float16)
        )  # [1024, 512]
        in_maps.append({"xT": x_c, "wT": wt, "biasT": bias_t})
    return in_maps


def _run(x, weights, bias_param, **spmd_kwargs):
    nc = _build()
    in_maps = _prep_inputs(x, weights, bias_param)
    res = run_bass_kernel_spmd(nc, in_maps, core_ids=list(range(N_CORES)), **spmd_kwargs)
    out = np.empty((BATCH, OUT_F), dtype=np.float32)
    for c in range(N_CORES):
        o = np.asarray(res.results[c]["outT"])  # [8, 128, 512] fp16
        out[c * B_CORE : (c + 1) * B_CORE] = (
            o.transpose(2, 0, 1).reshape(B_CORE, OUT_F).astype(np.float32)
        )
    return out, res


def kernel(x, weights, bias_param):
    out, _ = _run(x, weights, bias_param)
    return out


# revision 11
# speedup vs baseline: 1.6617x; 1.4712x over previous
"""KAN layer (Chebyshev order-7 on tanh(x)) as a Bass/Tile TRN2 kernel.

Math: out[b,o] = sum_{i,k} T_k(tanh(x[b,i])) * W[o,i,k] + bias[o],  k=0..7.

T_0 == 1, so the k=0 weight slice folds into an effective bias on the host:
bias_eff[o] = bias[o] + sum_i W[o,i,0]. The device contracts over the
remaining 7*1024 = 7168 (i,k) pairs, as 56 steps of K=128 (it-major).

Sharding: data-parallel over batch; each of the 8 cores takes 512 batch rows
and the full weights, producing out.T tiles [128(o) x 512(b)] in 8 PSUM banks.

Precision plan (gate is rel 2e-2; this lands ~1.4e-2):
- First 16 steps run in fp16 (full PE rate, 1 row/cycle).
- The last 40 steps run as 20 fp8e4m3 DoubleRow pairs (2 contraction steps
  per matmul at 0.5 cycles/row), each pair issued twice: once with the
  fp8-quantized weights w8 and once with the quantized residual
  wr = q8(w - w8). The residual recovers most of the weight-quantization
  error at DoubleRow cost; basis tiles are cast f16->f8 on-chip (the DVE/ACT
  cast is bit-exact vs ml_dtypes e4m3fn).
- All weights are pre-scaled by S=2^9 on the host so fp8 values sit in
  e4m3's normal band; the tail bias-add activation descales by 1/S.
- PSUM accumulates fp32 throughout; output is stored fp16 and cast to fp32
  on the host.
"""

import sys

sys.path.insert(0, "/opt/trn_rl_repo")

import numpy as np

import concourse.bass as bass  # noqa: F401  (engine types come via bacc)
import concourse.mybir as mybir
from concourse import bacc
from concourse.bass_utils import run_bass_kernel_spmd
from concourse.tile import TileContext

P = 128
N_CORES = 8
BATCH = 4096
B_CORE = BATCH // N_CORES  # 512
IN_F = 1024
OUT_F = 1024
KORD = 7  # Chebyshev T_1..T_7 (T_0 folded into bias)
N_ITILES = IN_F // P  # 8
N_OTILES = OUT_F // P  # 8
NSTEPS = N_ITILES * KORD  # 56 contraction steps of K=128
STEPS = [(it, k) for it in range(N_ITILES) for k in range(KORD)]
N16 = 16  # steps in fp16 (it0, it1, it2 k0-k1)
N_PAIRS = (NSTEPS - N16) // 2  # 20 fp8 DoubleRow pairs
N_TAILPAIRS = 3  # last pairs run otile-staggered so tails hide under PE
WSCALE = 2.0**9  # host weight pre-scale; descaled in the tail activation
NWARM = 56  # PE p-state warmup matmuls covering the DMA-latency head

F32 = mybir.dt.float32
F16 = mybir.dt.float16
F8 = mybir.dt.float8e4
ACT_TANH = mybir.ActivationFunctionType.Tanh
ACT_IDENT = mybir.ActivationFunctionType.Identity
ACT_COPY = mybir.ActivationFunctionType.Copy
MULT = mybir.AluOpType.mult
ADD = mybir.AluOpType.add
SUB = mybir.AluOpType.subtract
DR = mybir.MatmulPerfMode.DoubleRow

_NC_CACHE = None


def _build():
    """Build + compile the single-core Bass program (SPMD across 8 cores)."""
    global _NC_CACHE
    if _NC_CACHE is not None:
        return _NC_CACHE

    nc = bacc.Bacc("TRN2", target_bir_lowering=False, debug=False)

    # xT[i, b] = x[b, i] for this core's batch slice (fp16).
    xT = nc.declare_dram_parameter("xT", [IN_F, B_CORE], F16, isOutput=False)
    # fp16 weights (pre-scaled by WSCALE), per-partition k-major:
    # wf0/wf1 cover it0/it1 (7 k-tiles each), wf2 covers (it2, k0-k1).
    wf0 = nc.declare_dram_parameter("wf0", [P, KORD * OUT_F], F16, isOutput=False)
    wf1 = nc.declare_dram_parameter("wf1", [P, KORD * OUT_F], F16, isOutput=False)
    wf2 = nc.declare_dram_parameter("wf2", [P, 2 * OUT_F], F16, isOutput=False)
    # fp8 pair weights: [pair, p, l, {w8(1024) | wr(1024)}]
    w8T = nc.declare_dram_parameter(
        "w8T", [N_PAIRS, P, 2, 2 * OUT_F], F8, isOutput=False
    )
    # biasT[p, ot] = bias_eff[ot*128 + p] (unscaled)
    biasT = nc.declare_dram_parameter("biasT", [P, N_OTILES], F32, isOutput=False)
    # outT[ot, p, b] = out[b, ot*128 + p] (fp16; host casts to fp32)
    outT = nc.declare_dram_parameter("outT", [N_OTILES, P, B_CORE], F16, isOutput=True)

    pairs = [(STEPS[N16 + 2 * j], STEPS[N16 + 2 * j + 1]) for j in range(N_PAIRS)]

    with TileContext(nc) as tc:
        with (
            tc.tile_pool(name="basis", bufs=1) as basis_pool,
            tc.tile_pool(name="x", bufs=1) as x_pool,
            tc.tile_pool(name="w0", bufs=1) as w0_pool,
            tc.tile_pool(name="wbig", bufs=1) as wbig_pool,
            tc.tile_pool(name="w8", bufs=5) as w8_pool,
            tc.tile_pool(name="r8", bufs=1) as r8_pool,
            tc.tile_pool(name="chain", bufs=3) as chain_pool,
            tc.tile_pool(name="osb", bufs=8) as osb_pool,
            tc.tile_pool(name="misc", bufs=1) as misc_pool,
            tc.tile_pool(name="psum", bufs=1, space="PSUM") as psum_pool,
        ):
            psums = [
                psum_pool.tile([P, B_CORE], F32, name=f"ps_{ot}")
                for ot in range(N_OTILES)
            ]

            # -- t=0: scratch memset, dummy tanh (pulls the ACT table load
            #    off the critical path), PE warmup during the DMA-latency head.
            scratch = misc_pool.tile([P, 64], F16, name="scratch")
            nc.vector.memset(scratch, 0.0)
            dumout = misc_pool.tile([P, 1], F32, name="dumout")
            nc.scalar.activation(dumout, scratch[:, :1], ACT_TANH)

            bias_sb = misc_pool.tile([P, N_OTILES], F32, name="bias_sb")

            x_tiles = [
                x_pool.tile([P, B_CORE], F16, name=f"x_{it}")
                for it in range(N_ITILES)
            ]
            w0_tiles = [
                w0_pool.tile([P, OUT_F], F16, name=f"w0_{k}") for k in range(KORD)
            ]

            # x (+bias, +out stores) ride the SP HWDGE queue; every weight
            # fetch goes through the Pool SWDGE queue so the issue pipelines
            # parallelize and the head is gated only by x0+w00 arrival.
            H = B_CORE // 2
            nc.sync.dma_start(out=x_tiles[0][:, :H], in_=xT[0:P, :H])
            nc.sync.dma_start(out=x_tiles[0][:, H:], in_=xT[0:P, H:])
            for it in range(1, N_ITILES):
                nc.sync.dma_start(out=x_tiles[it], in_=xT[it * P : (it + 1) * P, :])
            nc.sync.dma_start(out=bias_sb, in_=biasT[:, :])

            for k in range(KORD):
                nc.gpsimd.dma_start(
                    out=w0_tiles[k], in_=wf0[:, k * OUT_F : (k + 1) * OUT_F]
                )
            w1_tile = wbig_pool.tile([P, KORD * OUT_F], F16, name="w1")
            nc.gpsimd.dma_start(out=w1_tile, in_=wf1[:, :])
            w2_tile = wbig_pool.tile([P, 2 * OUT_F], F16, name="w2")
            nc.gpsimd.dma_start(out=w2_tile, in_=wf2[:, :])
            w8_tiles = []
            for j in range(N_PAIRS):
                w8t = w8_pool.tile([P, 2, 2 * OUT_F], F8, tag="w8")
                nc.gpsimd.dma_start(out=w8t, in_=w8T[j, :, :, :])
                w8_tiles.append(w8t)

            # -- PE warmup: tiny self-matmuls on the zero scratch tile into a
            #    PSUM region later overwritten by the real start=True group.
            for i in range(NWARM):
                nc.tensor.matmul(
                    psums[0][:64, :64],
                    lhsT=scratch,
                    rhs=scratch,
                    start=True,
                    stop=True,
                )

            # -- Chebyshev basis in fp16: t1 = tanh(x); v = 2*t1;
            #    t2 = 2*t1^2 - 1; t_k = v*t_{k-1} - t_{k-2}.
            #    fp8 steps also get a f16->f8 cast into their pair tile
            #    (alternating ACT/DVE to balance engine load).
            r8_tiles = [
                r8_pool.tile([P, 2, B_CORE], F8, name=f"r8_{j}")
                for j in range(N_PAIRS)
            ]
            step_pos = {s: i for i, s in enumerate(STEPS)}

            def emit_cast(it, k, use_act):
                i = step_pos[(it, k)]
                if i < N16:
                    return
                j, l = divmod(i - N16, 2)
                dst = r8_tiles[j][:, l, :]
                if use_act:
                    nc.scalar.activation(dst, basis[it][k], ACT_COPY)
                else:
                    nc.vector.tensor_copy(dst, basis[it][k])

            basis = []  # basis[it][k'] = T_{k'+1} tile [128, 512] fp16
            for it in range(N_ITILES):
                t1 = basis_pool.tile([P, B_CORE], F16, name=f"b_{it}_0")
                if it == 0:
                    nc.scalar.activation(t1[:, :H], x_tiles[0][:, :H], ACT_TANH)
                    nc.scalar.activation(t1[:, H:], x_tiles[0][:, H:], ACT_TANH)
                else:
                    nc.scalar.activation(t1, x_tiles[it], ACT_TANH)
                v = chain_pool.tile([P, B_CORE], F16, tag="v")
                nc.vector.tensor_scalar_mul(v, t1, 2.0)
                m = chain_pool.tile([P, B_CORE], F16, tag="m")
                nc.vector.tensor_tensor(out=m, in0=t1, in1=t1, op=MULT)
                t2 = basis_pool.tile([P, B_CORE], F16, name=f"b_{it}_1")
                nc.vector.tensor_scalar(
                    out=t2, in0=m, scalar1=2.0, scalar2=-1.0, op0=MULT, op1=ADD
                )
                tiles = [t1, t2]
                for k in range(3, KORD + 1):
                    mk = chain_pool.tile([P, B_CORE], F16, tag="m")
                    nc.vector.tensor_tensor(out=mk, in0=v, in1=tiles[-1], op=MULT)
                    tk = basis_pool.tile([P, B_CORE], F16, name=f"b_{it}_{k - 1}")
                    nc.vector.tensor_tensor(out=tk, in0=mk, in1=tiles[-2], op=SUB)
                    tiles.append(tk)
                basis.append(tiles)
                for k in range(KORD):
                    emit_cast(it, k, use_act=(step_pos[(it, k)] % 2 == 0))

            # -- fp16 matmul steps -------------------------------------------
            def lhsT16_for(i, ot):
                it, k = STEPS[i]
                if it == 0:
                    return w0_tiles[k][:, ot * P : (ot + 1) * P]
                src = w1_tile if it == 1 else w2_tile
                col = k * OUT_F + ot * P
                return src[:, col : col + P]

            # Step 0 split into batch-halves: the first 8 matmuls only need
            # the first tanh half. start=True zeroes the whole 2KB
            # zero-region (bank), so the second half accumulates onto zeros.
            for half in range(2):
                sl = slice(0, H) if half == 0 else slice(H, B_CORE)
                for ot in range(N_OTILES):
                    nc.tensor.matmul(
                        psums[ot][:, sl], lhsT=lhsT16_for(0, ot),
                        rhs=basis[0][0][:, sl], start=(half == 0), stop=False,
                    )
            for i in range(1, N16):
                it, k = STEPS[i]
                rhs = basis[it][k]
                for ot in range(N_OTILES):
                    nc.tensor.matmul(
                        psums[ot], lhsT=lhsT16_for(i, ot), rhs=rhs,
                        start=False, stop=False,
                    )

            # -- fp8 DoubleRow pairs: w8 matmul + residual matmul ------------
            def dr_mms(j, ot, stop_last=False):
                for which in range(2):  # 0: w8, 1: wr
                    base = which * OUT_F + ot * P
                    nc.tensor.matmul(
                        psums[ot],
                        lhsT=w8_tiles[j][:, :, base : base + P],
                        rhs=r8_tiles[j],
                        start=False,
                        stop=(stop_last and which == 1),
                        perf_mode=DR,
                    )

            for j in range(N_PAIRS - N_TAILPAIRS):
                for ot in range(N_OTILES):
                    dr_mms(j, ot)

            # Final pairs otile-staggered: otile o finishes early enough for
            # its bias-add + store to hide under the remaining matmuls.
            for ot in range(N_OTILES):
                for j in range(N_PAIRS - N_TAILPAIRS, N_PAIRS):
                    dr_mms(j, ot, stop_last=(j == N_PAIRS - 1))
                osb = osb_pool.tile([P, B_CORE], F16, tag="osb")
                nc.scalar.activation(
                    osb, psums[ot], ACT_IDENT,
                    bias=bias_sb[:, ot : ot + 1], scale=1.0 / WSCALE,
                )
                nc.sync.dma_start(out=outT[ot, :, :], in_=osb)

    nc.compile()
    _NC_CACHE = nc
    return _NC_CACHE


def _prep_inputs(x, weights, bias_param):
    x = np.asarray(x, dtype=np.float32)
    weights = np.asarray(weights, dtype=np.float32)
    bias_param = np.asarray(bias_param, dtype=np.float32)
    f8np = mybir.dt.np(F8)

    # [o, i, k] -> per-itile [p, k, o], scaled by WSCALE
    w7 = weights.transpose(1, 2, 0)[:, 1:, :] * WSCALE  # [i, 7, o] fp32
    wit = w7.reshape(N_ITILES, P, KORD, OUT_F)  # [it, p, k, o]

    wf0 = np.ascontiguousarray(wit[0].reshape(P, KORD * OUT_F).astype(np.float16))
    wf1 = np.ascontiguousarray(wit[1].reshape(P, KORD * OUT_F).astype(np.float16))
    wf2 = np.ascontiguousarray(wit[2, :, :2].reshape(P, 2 * OUT_F).astype(np.float16))

    w8 = np.empty((N_PAIRS, P, 2, 2 * OUT_F), dtype=f8np)
    for j in range(N_PAIRS):
        for l in range(2):
            it, k = STEPS[N16 + 2 * j + l]
            w = wit[it, :, k, :]  # [128, 1024] fp32 (scaled)
            q = w.astype(f8np)
            r = (w - q.astype(np.float32)).astype(f8np)
            w8[j, :, l, :OUT_F] = q
            w8[j, :, l, OUT_F:] = r

    # exact T_0 fold in float64 (unscaled; applied after the 1/WSCALE tail)
    bias_eff = (
        bias_param.astype(np.float64) + weights[:, :, 0].astype(np.float64).sum(axis=1)
    ).astype(np.float32)
    bias_t = np.ascontiguousarray(bias_eff.reshape(N_OTILES, P).T)  # [128, 8]

    in_maps = []
    for c in range(N_CORES):
        x_c = np.ascontiguousarray(
            x[c * B_CORE : (c + 1) * B_CORE].T.astype(np.float16)
        )  # [1024, 512]
        in_maps.append(
            {"xT": x_c, "wf0": wf0, "wf1": wf1, "wf2": wf2, "w8T": w8, "biasT": bias_t}
        )
    return in_maps


def _run(x, weights, bias_param, **spmd_kwargs):
    nc = _build()
    in_maps = _prep_inputs(x, weights, bias_param)
    res = run_bass_kernel_spmd(nc, in_maps, core_ids=list(range(N_CORES)), **spmd_kwargs)
    out = np.empty((BATCH, OUT_F), dtype=np.float32)
    for c in range(N_CORES):
        o = np.asarray(res.results[c]["outT"])  # [8, 128, 512] fp16
        out[c * B_CORE : (c + 1) * B_CORE] = (
            o.transpose(2, 0, 1).reshape(B_CORE, OUT_F).astype(np.float32)
        )
    return out, res


def kernel(x, weights, bias_param):
    out, _ = _run(x, weights, bias_param)
    return out
